# revision 54
# baseline (speedup 1.0000x reference)
"""Trainium2 Bass kernel for nn_Estimation_88871463289511.

Math: the reference builds huge Kronecker matrices
  K_train_inv = invA (x) invB (x) invC   [5120,5120]
  K_test_train = A_tt (x) B (x) C        [Tq*256,5120]
but both only ever act on vectors, so every contraction factors per axis:
  Y[k,d,n] = sum_n' (C invC P)[n,n'] * (v @ kron(B invB, invA).T)[n',(d,k)]
  mean_xt[t,n,d] = sum_k G[t,k] Y[k,d,n]
with G[t,k] = dt * sum_{s<=idx_t} A_s[s,k] + rem_t * A_q[t,k] folding the
cumsum+gather into a 0/1 prefix-mask matmul (mask from a broadcast compare,
rem via a telescoping dts matmul). The device computes the RBF factors, the
searchsorted mask, G, and all query-time contractions; the host only
inverts/combines/packs the small replicated factor matrices (per the
sharding hint: "replicate the small factor matrices inv_A/inv_B/inv_C and v").

Sharding: data-parallel over the 64 query times -> 8 per core, SPMD on 8
cores; all factor inputs replicated, no collectives needed.

Single packed input (one DMA) mats [128, 307]:
  cols 0:40    vq2p = vsb @ kron(B invB, invA).T   (vsb[n,(d,k)] = v[k,n,d])
  cols 40:168  rT = (C @ invC @ P).T
  cols 168:171 [ ts_cmp | dts | tl(8,pad) ]  (ts_cmp poisoned at rows 0/S-1)
  cols 171:307 rows-block on partitions 0:2, regions
               P(0:100)=[ts;1] Q(100:120)=[1;-cs] R(120:128)=[tl;1] U(128:136)=[0;tl]
"""

import numpy as np

_NCORES = 8
K, N, D, S, T = 20, 128, 2, 100, 64
DK = D * K
TC = T // _NCORES
NOISE = 1e-4

_C0 = DK          # rT start
_C1 = DK + N      # cols start (168)
_C2 = _C1 + 3     # rows-block start (171)
_RCOLS = S + K + TC + TC  # 136
_MCOLS = _C2 + _RCOLS  # 307

_CACHE = {}


def _build(a, ln_dt):
    import concourse.bacc as bacc
    import concourse.mybir as mybir
    from concourse import masks, tile
    from concourse.mybir import ActivationFunctionType as act
    from concourse.mybir import AluOpType as alu

    f32 = mybir.dt.float32
    nc = bacc.Bacc("TRN2", target_bir_lowering=False, debug=False,
                   num_devices=_NCORES)

    mats_d = nc.dram_tensor("mats", [N, _MCOLS], f32, kind="ExternalInput")
    out_d = nc.dram_tensor("out", [TC, N, D], f32, kind="ExternalOutput")

    with tile.TileContext(nc) as tc:
        with (
            tc.tile_pool(name="sb", bufs=1) as sb,
            tc.tile_pool(name="ps", bufs=1, space="PSUM") as ps,
        ):
            mats = sb.tile([N, _MCOLS], f32)
            nc.sync.dma_start(mats[:], mats_d[:])
            vq2p = mats[:, 0:DK]
            rT = mats[:, _C0 : _C0 + N]
            ts_col = mats[0:S, _C1 : _C1 + 1]
            dts_col = mats[0:S, _C1 + 1 : _C1 + 2]
            tl_col = mats[0:TC, _C1 + 2 : _C1 + 3]
            r = _C2
            ts_ones = mats[0:2, r : r + S]                  # [2,100]: ts / 1
            ones_cneg = mats[0:2, r + S : r + S + K]        # [2,20]:  1 / -cs
            tl_ones = mats[0:2, r + S + K : r + S + K + TC]  # [2,8]: tl / 1
            zero_tl = mats[0:2, r + S + K + TC : r + _RCOLS]  # [2,8]: 0 / tl

            # per-partition ln(dt) bias column + small identity, on idle Pool
            lnd_col = sb.tile([S, 1], f32)
            nc.gpsimd.memset(lnd_col[:], ln_dt)
            i8 = sb.tile([TC, TC], f32)
            masks.make_identity(nc, i8[:])

            # ---- prefix mask Mt[s,t] = 1{s <= idx_t} ----
            # ts_cmp is poisoned host-side: row 0 -> -1e15 (always 1) and
            # row S-1 -> +1e15 (always 0; searchsorted caps idx at S-2).
            tlb_ps = ps.tile([S, TC], f32, tag="mask")
            nc.tensor.matmul(tlb_ps[:], ts_ones, zero_tl, start=True, stop=True)
            mt = sb.tile([S, TC], f32)
            nc.vector.tensor_single_scalar(mt[:], tlb_ps[:], ts_col, alu.is_ge)

            # ---- RBF factors: diff via one 2-row matmul each, square, exp ----
            # (A_s row S-1 underflows to 0 via the poisoned ts)
            ds_ps = ps.tile([S, K], f32, tag="bc", bufs=2)
            nc.tensor.matmul(ds_ps[:], ts_ones, ones_cneg, start=True, stop=True)
            sq_s = sb.tile([S, K], f32)
            nc.scalar.square(sq_s[:], ds_ps[:])
            as_sb = sb.tile([S, K], f32)  # dt * exp(a (ts - c)^2)
            nc.scalar.activation(as_sb[:], sq_s[:], act.Exp, bias=lnd_col[:], scale=a)
            dq_ps = ps.tile([TC, K], f32, tag="bc", bufs=2)
            nc.tensor.matmul(dq_ps[:], tl_ones, ones_cneg, start=True, stop=True)
            sq_q = sb.tile([TC, K], f32)
            nc.scalar.square(sq_q[:], dq_ps[:])
            aq_sb = sb.tile([TC, K], f32)  # exp(a (tl - c)^2)
            nc.scalar.activation(aq_sb[:], sq_q[:], act.Exp, scale=a)

            # ---- rem = tl - ts[idx] (telescoping via dts) ----
            tsat_ps = ps.tile([TC, 1], f32, tag="fin", bufs=2)
            nc.tensor.matmul(tsat_ps[:], mt[:], dts_col, start=True, stop=True)
            rem = sb.tile([TC, 1], f32)
            nc.vector.tensor_sub(rem[:], tl_col, tsat_ps[:])
            aqr = sb.tile([TC, K], f32)
            nc.vector.tensor_scalar_mul(aqr[:], aq_sb[:], rem[:])

            # ---- Gt[k,t] = As^T Mt + (Aq*rem)^T ----
            gt_ps = ps.tile([K, TC], f32, tag="fin", bufs=2)
            nc.tensor.matmul(gt_ps[:], as_sb[:], mt[:], start=True, stop=False)
            nc.tensor.matmul(gt_ps[:], aqr[:], i8[:], start=False, stop=True)
            gt_sb = sb.tile([K, TC], f32)
            nc.vector.tensor_copy(gt_sb[:], gt_ps[:])

            # ---- Y: one matmul per d off the packed host-combined factors ----
            up = ps.tile([K, 2 * N], f32, tag="chain")
            nc.tensor.matmul(up[:, 0:N], vq2p[:, 0:K], rT, start=True, stop=True)
            nc.tensor.matmul(up[:, N : 2 * N], vq2p[:, K:DK], rT, start=True, stop=True)
            # interleave (d n) -> (n d) here, off the critical tail
            us = sb.tile([K, 2 * N], f32)  # us[k, n*D+d] = Y[k,d,n]
            nc.vector.tensor_copy(
                us[:].rearrange("k (n d) -> k n d", n=N, d=D),
                up[:].rearrange("k (d n) -> k d n", d=D, n=N).transpose([0, 2, 1]),
            )

            # ---- out[t, n*D+d] = sum_k Gt[k,t] us[k, n*D+d] ----
            o_ps = ps.tile([TC, D * N], f32, tag="out")
            nc.tensor.matmul(o_ps[:], gt_sb[:], us[:], start=True, stop=True)
            o_sb = sb.tile([TC, N * D], f32)
            nc.vector.tensor_copy(o_sb[:], o_ps[:])
            nc.sync.dma_start(out_d[:].rearrange("t n d -> t (n d)"), o_sb[:])

    nc.compile()
    return nc


def _inv_like_reference(A, B, C):
    """f32 inverses via jax-on-CPU (XLA LAPACK path) to track the reference's
    jnp.linalg.inv bit-for-bit; falls back to numpy if jax is unavailable."""
    try:
        import jax

        with jax.default_device(jax.devices("cpu")[0]):
            import jax.numpy as jnp

            return tuple(np.asarray(jnp.linalg.inv(jnp.asarray(m)))
                         for m in (A, B, C))
    except Exception:
        return tuple(np.linalg.inv(m).astype(np.float32) for m in (A, B, C))


def _prepare(times_list, time_samples, bin_bounds, v, B_mat, C_mat, sigma):
    f32 = np.float32
    tl = np.ascontiguousarray(np.asarray(times_list, dtype=f32))
    ts = np.ascontiguousarray(np.asarray(time_samples, dtype=f32))
    bb = np.asarray(bin_bounds, dtype=f32)
    v = np.ascontiguousarray(np.asarray(v, dtype=f32))
    B = np.asarray(B_mat, dtype=f32)
    C = np.ascontiguousarray(np.asarray(C_mat, dtype=f32))
    sig = float(np.asarray(sigma))

    cs = (0.5 * (bb[1:] + bb[:-1])).astype(f32)
    dcc = (cs[:, None] - cs[None, :]) / f32(sig)
    A_train = np.exp(f32(-0.5) * dcc * dcc, dtype=f32) + f32(NOISE) * np.eye(K, dtype=f32)
    invA, invB, invC = _inv_like_reference(A_train, B, C)
    P = np.eye(N, dtype=f32) - np.full((N, N), 1.0 / N, dtype=f32)
    R = (C @ (invC @ P)).astype(f32)
    q2 = np.kron(B @ invB, invA).astype(f32)
    dts = np.empty(S, f32)
    dts[0] = ts[0]
    dts[1:] = np.diff(ts)
    dt = float(ts[1] - ts[0])
    a = -0.5 / (sig * sig)
    ln_dt = float(np.log(dt))
    # poisoned copy for the RBF rows / mask compare
    tsq = ts.copy()
    tsq[S - 1] = 1e15
    ts_cmp = tsq.copy()
    ts_cmp[0] = -1e15

    vsb = np.transpose(v, (1, 2, 0)).reshape(N, DK)  # [n, (d k)]
    vq2p = (vsb @ q2.T).astype(f32)  # [n, (d k)] with invA/BinvB applied

    mats = np.zeros((N, _MCOLS), f32)
    mats[:, 0:DK] = vq2p
    mats[:, _C0 : _C0 + N] = R.T
    mats[0:S, _C1] = ts_cmp
    mats[0:S, _C1 + 1] = dts
    r = _C2
    mats[0, r : r + S] = tsq
    mats[1, r : r + S] = 1.0
    mats[0, r + S : r + S + K] = 1.0
    mats[1, r + S : r + S + K] = -cs
    mats[1, r + S + K : r + S + K + TC] = 1.0

    in_maps = []
    for c in range(_NCORES):
        tlc = tl[c * TC : (c + 1) * TC]
        m = mats.copy()
        m[0:TC, _C1 + 2] = tlc
        m[0, r + S + K : r + S + K + TC] = tlc
        m[1, r + S + K + TC : r + _RCOLS] = tlc
        in_maps.append(dict(mats=np.ascontiguousarray(m)))
    return a, ln_dt, in_maps


def _run(inputs, trace=False):
    from concourse.bass_utils import run_bass_kernel_spmd

    a, ln_dt, in_maps = _prepare(**inputs)
    key = (round(a, 12), round(ln_dt, 12))
    if key not in _CACHE:
        _CACHE[key] = _build(a, ln_dt)
    nc = _CACHE[key]
    res = run_bass_kernel_spmd(nc, in_maps, list(range(_NCORES)), trace=trace)
    out = np.concatenate([res.results[c]["out"] for c in range(_NCORES)], axis=0)
    return out.astype(np.float32), res


def kernel(**inputs):
    out, _ = _run(inputs, trace=False)
    return out
